# revision 15
# baseline (speedup 1.0000x reference)
"""4-layer GAT (PyG GATConv semantics) on 8 Trainium2 NeuronCores via Bass/Tile.

Sharding: nodes are split into 8 contiguous ranges (dst-partitioning); each
core owns the edges whose dst lands in its range (graph partitioning done on
host).  Per layer:
  1. dense phase (per core, own nodes): xp = h @ W' (BN scale folded) in bf16,
     es/ed head scores via host-precombined matrices; rows
     [xp(bf16) || es(f32-in-bf16-pairs) || ed(bf16) || pad] -> DRAM table.
  2. AllGather the table (chunked, overlapped with the edge phase tail).
  3. edge phase: per 128-node dst group, bulk dma_gather of src rows (int16
     indices; 4 SWDGE queues round-robin; src split into two 32K halves),
     per-edge softmax numerators z = exp(leaky_relu(es+ed)) on the scalar
     engine, and a 0/1-indicator bf16 matmul (is_equal against iota, computed
     on-chip) that segment-sums z*xp and z into PSUM; divide by the z-sum,
     add folded BN shift, ReLU.
Final: graph mean-pool via the same indicator-matmul trick, AllReduce the
[64, HC] partials, then the tiny MLP + sigmoid on every core.
"""

import ml_dtypes
import numpy as np

P = 128
N_GRAPHS = 64
NEG_SLOPE = 0.2
BN_EPS = 1e-5
NC_BLK = 8       # edge chunks (of 128 edges) per dma_gather call
HALF = 32768     # int16 index range per table half
AG_CHUNKS = 5    # allgather chunks (uneven; small tail)

# full-size problem constants (hardcoded per harness contract)
FULL = dict(N=50000, FIN=128, H=4, C=64, n_cores=8)


def _ceil(a, b):
    return -(-a // b)


# ----------------------------------------------------------------------------
# host-side preprocessing
# ----------------------------------------------------------------------------

def preprocess(inputs, N, FIN, H, C, n_cores):
    """Partition graph + fold BN + build per-core device input maps."""
    HC = H * C
    ROWP = 512                         # fp8 elems (bytes) per table row
    npc = N // n_cores
    ngrp = _ceil(npc, P)

    ei = np.asarray(inputs["edge_index"])
    loops = np.arange(N, dtype=np.int64)
    src = np.concatenate([ei[0], loops]).astype(np.int64)
    dst = np.concatenate([ei[1], loops]).astype(np.int64)

    order = np.argsort(dst, kind="stable")
    src_s, dst_s = src[order], dst[order]
    core_of = dst_s // npc
    loc = dst_s - core_of * npc
    grp_of = loc // P
    # remap src node ids to the AG-chunk-major table layout:
    # table rows = [chunk0: core0 seg, .., core7 seg | chunk1: ...]
    seg = npc // AG_CHUNKS
    k_of = src_s // npc
    r_of = src_s - k_of * npc
    ch_of = r_of // seg
    src_s = ch_of * (n_cores * seg) + k_of * seg + (r_of - ch_of * seg)
    is_high = src_s >= HALF

    # per (core, group): low-src edges then high-src edges, each padded to
    # a multiple of 128; chunk counts maxed across cores (SPMD program).
    cntl = np.zeros((n_cores, ngrp), dtype=np.int64)
    cnth = np.zeros((n_cores, ngrp), dtype=np.int64)
    np.add.at(cntl, (core_of[~is_high], grp_of[~is_high]), 1)
    np.add.at(cnth, (core_of[is_high], grp_of[is_high]), 1)
    nchl = np.maximum(1, _ceil(cntl, P).max(axis=0))
    nchh = _ceil(cnth, P).max(axis=0)
    if N <= HALF:
        nchh[:] = 0
    nch = nchl + nchh
    chunk_base = np.concatenate([[0], np.cumsum(nch)]).astype(np.int64)
    TC = int(chunk_base[-1])

    src16 = np.zeros((n_cores, 16, TC * 8), dtype=np.int16)
    dstl_arr = np.full((n_cores, P, TC), -1.0, dtype=np.float32)

    def place(k, g, edges_src, edges_dst, base_chunk, high):
        """edges sorted by dst; slot j -> (p=j%128, chunk=base+j//128)."""
        n_e = edges_src.shape[0]
        if n_e == 0:
            return
        j = np.arange(n_e)
        p_idx = j % P
        c_idx = base_chunk + j // P
        v = edges_src - (HALF if high else 0)
        src16[k, p_idx % 16, c_idx * 8 + p_idx // 16] = v.astype(np.int16)
        dstl_arr[k, p_idx, c_idx] = edges_dst.astype(np.float32)

    for k in range(n_cores):
        m_core = core_of == k
        sk, dk, gk, hk = (src_s[m_core], loc[m_core], grp_of[m_core],
                          is_high[m_core])
        for g in range(ngrp):
            m = gk == g
            sg, dg, hg = sk[m], dk[m] - g * P, hk[m]
            cb = int(chunk_base[g])
            place(k, g, sg[~hg], dg[~hg], cb, False)
            if nchh[g]:
                place(k, g, sg[hg], dg[hg], cb + int(nchl[g]), True)
    src16 = np.tile(src16, (1, 8, 1))   # replicate across 16-partition groups

    # node-partition indicator M[p=node, c, e], shipped bf16 (edp matmul lhsT)
    nrng = np.arange(P, dtype=np.float32)
    m_ship = np.zeros((n_cores, P, TC * P), dtype=ml_dtypes.float8_e4m3fn)
    for k in range(n_cores):
        eq = (dstl_arr[k][:, :, None] == nrng[None, None, :])  # [e, c, n]
        m_ship[k] = np.ascontiguousarray(
            eq.transpose(2, 1, 0)).astype(ml_dtypes.float8_e4m3fn).reshape(
                P, TC * P)

    # pooling batch ids per (p, g); -1 for pad nodes
    batch = np.asarray(inputs["batch"]).astype(np.int64)
    bat_arr = np.full((n_cores, P, ngrp), -1.0, dtype=np.float32)
    for k in range(n_cores):
        bk = batch[k * npc:(k + 1) * npc]
        for g in range(ngrp):
            rows = bk[g * P:(g + 1) * P]
            bat_arr[k, :rows.shape[0], g] = rows.astype(np.float32)

    # fold BN into weights
    wcats, treps = [], []
    dins = [FIN, HC, HC, HC]
    for l in range(1, 5):
        W = np.asarray(inputs[f"W{l}"], np.float32)
        a_s = np.asarray(inputs[f"as{l}"], np.float32)
        a_d = np.asarray(inputs[f"ad{l}"], np.float32)
        b = np.asarray(inputs[f"b{l}"], np.float32)
        g_ = np.asarray(inputs[f"g{l}"], np.float32)
        be = np.asarray(inputs[f"be{l}"], np.float32)
        rm = np.asarray(inputs[f"rm{l}"], np.float32)
        rv = np.asarray(inputs[f"rv{l}"], np.float32)
        S = g_ / np.sqrt(rv + BN_EPS)
        T = (b - rm) * S + be
        Wp = W * S[None, :]
        Wr = W.reshape(dins[l - 1], H, C)
        Aes = np.einsum("dhc,hc->dh", Wr, a_s).astype(np.float32)
        Aed = np.einsum("dhc,hc->dh", Wr, a_d).astype(np.float32)
        wcats.append(np.concatenate([Wp, Aes, Aed], axis=1).astype(
            ml_dtypes.bfloat16))
        treps.append(np.tile(T[None, :], (P, 1)).astype(np.float32))

    x = np.asarray(inputs["x"], np.float32)
    iota = np.tile(np.arange(P, dtype=np.float32)[None, :], (P, 1))
    ident = np.eye(P, dtype=ml_dtypes.bfloat16)
    cntf = np.bincount(batch, minlength=N_GRAPHS).astype(np.float32)
    rcinv = (1.0 / np.clip(cntf, 1.0, None)).reshape(N_GRAPHS, 1).astype(
        np.float32)
    wm1 = np.asarray(inputs["Wm1"], np.float32).astype(ml_dtypes.bfloat16)
    bm1 = np.tile(np.asarray(inputs["bm1"], np.float32)[None, :],
                  (N_GRAPHS, 1))
    wm2 = np.asarray(inputs["Wm2"], np.float32).astype(ml_dtypes.bfloat16)
    bm2 = np.tile(np.asarray(inputs["bm2"], np.float32)[None, :],
                  (N_GRAPHS, 1))

    meta = dict(
        N=N, FIN=FIN, H=H, C=C, HC=HC, ROWP=ROWP, npc=npc, ngrp=ngrp,
        nchl=[int(v) for v in nchl], nchh=[int(v) for v in nchh],
        chunk_base=[int(v) for v in chunk_base], TC=TC, n_cores=n_cores,
    )

    in_maps = []
    for k in range(n_cores):
        m = dict(
            xT=np.ascontiguousarray(np.pad(
                x[k * npc:(k + 1) * npc], ((0, ngrp * P - npc), (0, 0))
                ).T).astype(ml_dtypes.bfloat16),
            src16=np.ascontiguousarray(src16[k]),
            dstl=np.ascontiguousarray(dstl_arr[k]),
            m_ship=m_ship[k],
            batchf=np.ascontiguousarray(bat_arr[k]),
            iota=iota, ident=ident, rcinv=rcinv,
            wm1=wm1, bm1=bm1, wm2=wm2, bm2=bm2,
        )
        for l in range(1, 5):
            m[f"wcat{l}"] = wcats[l - 1]
            m[f"trep{l}"] = treps[l - 1]
        in_maps.append(m)
    return meta, in_maps


# ----------------------------------------------------------------------------
# bass program
# ----------------------------------------------------------------------------

def build_bass(meta):
    import concourse.bacc as bacc
    import concourse.bass as bass
    import concourse.mybir as mybir
    import concourse.tile as tile
    from contextlib import ExitStack

    f32 = mybir.dt.float32
    f32r = mybir.dt.float32r
    bf16 = mybir.dt.bfloat16
    f8 = mybir.dt.float8e4
    i16 = mybir.dt.int16
    Alu = mybir.AluOpType
    Act = mybir.ActivationFunctionType

    N, FIN, H, C, HC = meta["N"], meta["FIN"], meta["H"], meta["C"], meta["HC"]
    ROWP, npc, ngrp, TC = meta["ROWP"], meta["npc"], meta["ngrp"], meta["TC"]
    nchl, nchh, chunk_base = meta["nchl"], meta["nchh"], meta["chunk_base"]
    n_cores = meta["n_cores"]
    DROW = HC + 2 * H         # dense-phase psum row (f32): xp | es | ed
    VW = HC + H               # scatter rhs row: values | z
    OFFES = HC                # es (raw f32) at row bytes [HC, HC+4H)
    OFFED = HC + 4 * H        # ed (raw bf16) at row bytes [HC+4H, HC+6H)
    RUSED = HC + 6 * H        # used row bytes
    dins = [FIN, HC, HC, HC]
    MAXC = max(nchl[g] + nchh[g] for g in range(ngrp))
    RG = [list(range(n_cores))]

    nc = bacc.Bacc("TRN2", target_bir_lowering=False, debug=False,
                   num_devices=n_cores, num_swdge_queues=4)

    # I/O
    t_x = nc.dram_tensor("xT", [FIN, ngrp * P], bf16, kind="ExternalInput")
    t_s16 = nc.dram_tensor("src16", [P, TC * 8], i16, kind="ExternalInput")
    t_dstl = nc.dram_tensor("dstl", [P, TC], f32, kind="ExternalInput")
    t_m = nc.dram_tensor("m_ship", [P, TC * P], f8, kind="ExternalInput")
    t_bat = nc.dram_tensor("batchf", [P, ngrp], f32, kind="ExternalInput")
    t_iota = nc.dram_tensor("iota", [P, P], f32, kind="ExternalInput")
    t_ident = nc.dram_tensor("ident", [P, P], bf16, kind="ExternalInput")
    t_rcinv = nc.dram_tensor("rcinv", [N_GRAPHS, 1], f32, kind="ExternalInput")
    t_wcat = [nc.dram_tensor(f"wcat{l}", [dins[l - 1], DROW], bf16,
                             kind="ExternalInput") for l in range(1, 5)]
    t_trep = [nc.dram_tensor(f"trep{l}", [P, HC], f32, kind="ExternalInput")
              for l in range(1, 5)]
    t_wm1 = nc.dram_tensor("wm1", [HC, 32], bf16, kind="ExternalInput")
    t_bm1 = nc.dram_tensor("bm1", [N_GRAPHS, 32], f32, kind="ExternalInput")
    t_wm2 = nc.dram_tensor("wm2", [32, 1], bf16, kind="ExternalInput")
    t_bm2 = nc.dram_tensor("bm2", [N_GRAPHS, 1], f32, kind="ExternalInput")
    t_out = nc.dram_tensor("out", [N_GRAPHS, 1], f32, kind="ExternalOutput")

    cc_in = [nc.dram_tensor(f"cc_in{l}", [npc, ROWP], f8)
             for l in range(1, 5)]
    table = [nc.dram_tensor(f"table{l}", [N, ROWP], f8, addr_space="Shared")
             for l in range(1, 5)]
    ar_in = nc.dram_tensor("ar_in", [N_GRAPHS, HC], f32)
    ar_out = nc.dram_tensor("ar_out", [N_GRAPHS, HC], f32,
                            addr_space="Shared")

    # allgather row chunking (rows per core per chunk); smaller tail
    fr = [0.0, 0.3, 0.55, 0.75, 0.9, 1.0]
    ag_edges = [int(round(npc * f)) for f in fr]
    # group index after whose dense phase ag chunk i may start
    ag_grp = [_ceil(ag_edges[i + 1], P) - 1 for i in range(AG_CHUNKS)]

    with tile.TileContext(nc) as tc, ExitStack() as ctx:
        cpool = ctx.enter_context(tc.tile_pool(name="consts", bufs=1))
        wpool = ctx.enter_context(tc.tile_pool(name="weights", bufs=1))
        work = ctx.enter_context(tc.tile_pool(name="work", bufs=3))
        gpool = ctx.enter_context(tc.tile_pool(name="gath", bufs=6))
        mpool = ctx.enter_context(tc.tile_pool(name="mmp", bufs=2))
        psum = ctx.enter_context(tc.tile_pool(name="psum", bufs=2,
                                              space="PSUM"))
        gps_pool = ctx.enter_context(tc.tile_pool(name="gpsum", bufs=2,
                                                  space="PSUM"))

        # ---- constants ----
        iota_t = cpool.tile([P, P], f32, tag="iota")
        ident_t = cpool.tile([P, P], bf16, tag="ident")
        s16_t = cpool.tile([P, TC * 8], i16, tag="s16")
        dstl_t = cpool.tile([P, TC], f32, tag="dstl")
        bat_t = cpool.tile([P, ngrp], f32, tag="bat")
        for tt, src_dram in ((iota_t, t_iota), (ident_t, t_ident),
                             (s16_t, t_s16), (dstl_t, t_dstl),
                             (bat_t, t_bat)):
            nc.sync.dma_start(out=tt[:], in_=src_dram[:])

        wcat_t = []
        for l in range(4):
            din = dins[l]
            ks_t = []
            for ks in range(_ceil(din, P)):
                ksz = min(P, din - ks * P)
                wt = wpool.tile([ksz, DROW], bf16, tag=f"wcat{l}_{ks}")
                nc.sync.dma_start(out=wt[:],
                                  in_=t_wcat[l][ks * P:ks * P + ksz, :])
                ks_t.append(wt)
            wcat_t.append(ks_t)
        trep_t = []
        for l in range(4):
            tt = wpool.tile([P, HC], f32, tag=f"trep{l}")
            nc.sync.dma_start(out=tt[:], in_=t_trep[l][:])
            trep_t.append(tt)
        wm1_t = []
        for ks in range(_ceil(HC, P)):
            ksz = min(P, HC - ks * P)
            wt = wpool.tile([ksz, 32], bf16, tag=f"wm1_{ks}")
            nc.sync.dma_start(out=wt[:], in_=t_wm1[ks * P:ks * P + ksz, :])
            wm1_t.append(wt)
        rcinv_t = wpool.tile([N_GRAPHS, 1], f32, tag="rcinv")
        bm1_t = wpool.tile([N_GRAPHS, 32], f32, tag="bm1")
        wm2_t = wpool.tile([32, 1], bf16, tag="wm2")
        bm2_t = wpool.tile([N_GRAPHS, 1], f32, tag="bm2")
        nc.sync.dma_start(out=rcinv_t[:], in_=t_rcinv[:])
        nc.sync.dma_start(out=bm1_t[:], in_=t_bm1[:])
        nc.sync.dma_start(out=wm2_t[:], in_=t_wm2[:])
        nc.sync.dma_start(out=bm2_t[:], in_=t_bm2[:])

        pool_in = cpool.tile([P, ngrp, HC], bf16, tag="poolin")

        qctr = [0]   # SWDGE queue round-robin counter

        def dense(l, g, hT):
            """layer-(l+1) dense for group g; hT = list of [ksz, m] lhsT."""
            din = dins[l]
            nks = _ceil(din, P)
            gpn = min(npc - g * P, P)
            dp = psum.tile([P, DROW], f32, tag="dens")
            for ks in range(nks):
                ksz = min(P, din - ks * P)
                nc.tensor.matmul(dp[:], lhsT=hT[ks],
                                 rhs=wcat_t[l][ks][:ksz, :],
                                 start=(ks == 0), stop=(ks == nks - 1))
            st = work.tile([P, RUSED], f8, tag="stage")
            nc.scalar.activation(st[:, 0:HC], dp[:, 0:HC], Act.Copy)
            nc.vector.tensor_copy(st[:, OFFES:OFFES + 4 * H].bitcast(f32),
                                  dp[:, HC:HC + H])
            nc.scalar.activation(st[:, OFFED:OFFED + 2 * H].bitcast(bf16),
                                 dp[:, HC + H:DROW], Act.Copy)
            nc.sync.dma_start(out=cc_in[l][g * P:g * P + gpn, 0:RUSED],
                              in_=st[:gpn, 0:RUSED])

        def edge(l, g):
            """edge phase of layer l (1-based) for dst group g."""
            n_c = nchl[g] + nchh[g]
            cb = chunk_base[g]
            gpn = min(npc - g * P, P)
            # ed for own nodes of this group, from local staging (bf16)
            edt = work.tile([P, H], bf16, tag="edt")
            if gpn < P:
                nc.vector.memset(edt[:], 0.0)
            nc.sync.dma_start(
                out=edt[:gpn, :],
                in_=cc_in[l - 1][g * P:g * P + gpn,
                                 OFFED:OFFED + 2 * H].bitcast(bf16))
            gps = gps_pool.tile([P, VW], f32, tag="grp")
            mmg = mpool.tile([P, MAXC, P], f8, tag="mm")
            nc.sync.dma_start(out=mmg[:, :n_c, :],
                              in_=t_m[:, cb * P:(cb + n_c) * P])
            blocks = []
            for b0 in range(0, nchl[g], NC_BLK):
                blocks.append((b0, min(NC_BLK, nchl[g] - b0), False))
            for b0 in range(nchl[g], n_c, NC_BLK):
                blocks.append((b0, min(NC_BLK, n_c - b0), True))
            for b0, nbc, high in blocks:
                c0 = cb + b0
                gt = gpool.tile([P, NC_BLK, ROWP], f8, tag="gath")
                in_ap = table[l - 1][HALF:, :] if high else table[l - 1][:]
                nc.gpsimd.dma_gather(
                    out_ap=gt[:, :nbc, :], in_ap=in_ap,
                    idxs_ap=s16_t[:, c0 * 8:(c0 + nbc) * 8],
                    num_idxs=nbc * P, num_idxs_reg=nbc * P,
                    elem_size=ROWP, queue_num=qctr[0] & 3,
                )
                qctr[0] += 1
                # on-chip dst indicator: mtb[e, c, n] = (dstl[e, c] == n)
                mtb = gpool.tile([P, NC_BLK, P], f8, tag="mtb")
                nc.vector.tensor_tensor(
                    out=mtb[:, :nbc, :],
                    in0=dstl_t[:, c0:c0 + nbc].unsqueeze(2).to_broadcast(
                        [P, nbc, P]),
                    in1=iota_t[:].unsqueeze(1).to_broadcast([P, nbc, P]),
                    op=Alu.is_equal)
                # ed broadcast node->edges via node-partition indicator
                edp = psum.tile([P, NC_BLK * H], f32, tag="edp")
                for c in range(nbc):
                    nc.tensor.matmul(edp[:, c * H:(c + 1) * H],
                                     lhsT=mmg[:, b0 + c, :], rhs=edt[:],
                                     start=True, stop=True)
                sc = work.tile([P, NC_BLK * H], f32, tag="sc")
                es_ap = gt[:, :nbc, OFFES:OFFES + 4 * H].bitcast(f32)
                nc.vector.tensor_tensor(
                    out=sc[:, 0:nbc * H].rearrange("p (n h) -> p n h", h=H),
                    in0=es_ap,
                    in1=edp[:, 0:nbc * H].rearrange("p (n h) -> p n h", h=H),
                    op=Alu.add)
                e1 = work.tile([P, NC_BLK * H], f32, tag="e1")
                nc.scalar.activation(e1[:, 0:nbc * H], sc[:, 0:nbc * H],
                                     Act.Exp)
                e2 = work.tile([P, NC_BLK * H], f32, tag="e2")
                nc.scalar.activation(e2[:, 0:nbc * H], sc[:, 0:nbc * H],
                                     Act.Exp, scale=NEG_SLOPE)
                vt = gpool.tile([P, NC_BLK, VW], bf16, tag="vt")
                zs = vt[:, :nbc, HC:HC + H]
                nc.vector.tensor_tensor(
                    out=zs,
                    in0=e1[:, 0:nbc * H].rearrange("p (n h) -> p n h", h=H),
                    in1=e2[:, 0:nbc * H].rearrange("p (n h) -> p n h", h=H),
                    op=Alu.max)
                nc.vector.tensor_tensor(
                    out=vt[:, :nbc, 0:HC].rearrange(
                        "p n (h c) -> p n h c", c=C),
                    in0=gt[:, :nbc, 0:HC].rearrange(
                        "p n (h c) -> p n h c", c=C),
                    in1=zs.unsqueeze(3).to_broadcast([P, nbc, H, C]),
                    op=Alu.mult,
                )
                for c in range(nbc):
                    nc.tensor.matmul(
                        gps[:], lhsT=mtb[:, c, :], rhs=vt[:, c, :],
                        start=(b0 + c == 0), stop=(b0 + c == n_c - 1),
                    )
            # postprocess: divide by z-sum, add BN shift, relu (layers 1-3)
            d4 = work.tile([P, H], f32, tag="d4")
            nc.vector.tensor_scalar_max(d4[:], gps[:, HC:HC + H], 1e-30)
            r4 = work.tile([P, H], f32, tag="r4")
            nc.vector.reciprocal(r4[:], d4[:])
            ht = work.tile([P, HC], f32, tag="hbuf")
            nc.vector.tensor_tensor(
                out=ht[:].rearrange("p (h c) -> p h c", c=C),
                in0=gps[:, 0:HC].rearrange("p (h c) -> p h c", c=C),
                in1=r4[:].unsqueeze(2).to_broadcast([P, H, C]),
                op=Alu.mult)
            if l < 4:
                nc.vector.tensor_tensor(out=ht[:], in0=ht[:],
                                        in1=trep_t[l - 1][:], op=Alu.add)
                hb = work.tile([P, HC], bf16, tag="hrelu")
                nc.scalar.activation(hb[:], ht[:], Act.Relu)
                hT = []
                for ks in range(_ceil(HC, P)):
                    tp = psum.tile([P, P], bf16, tag="transp")
                    nc.tensor.transpose(tp[:], hb[:, ks * P:(ks + 1) * P],
                                        ident_t[:])
                    htt = work.tile([P, P], bf16, tag=f"hT{ks}")
                    nc.scalar.activation(htt[:], tp[:], Act.Copy)
                    hT.append(htt[:])
                dense(l, g, hT)
            else:
                nc.vector.tensor_tensor(out=pool_in[:, g, 0:HC], in0=ht[:],
                                        in1=trep_t[l - 1][:], op=Alu.add)

        def allgather(l, i):
            r0, r1 = ag_edges[i], ag_edges[i + 1]
            nc.gpsimd.collective_compute(
                "AllGather", Alu.bypass, replica_groups=RG,
                ins=[cc_in[l][r0:r1, :].opt()],
                outs=[table[l][r0 * n_cores:r1 * n_cores, :].opt()],
            )

        # ---- program ----
        # layer-1 dense over own nodes (x pre-transposed), AG interleaved
        xT_t = cpool.tile([FIN, ngrp * P], bf16, tag="xT")
        nc.sync.dma_start(out=xT_t[:], in_=t_x[:])
        agi = 0
        for g in range(ngrp):
            gpn = min(npc - g * P, P)
            dense(0, g, [xT_t[:, g * P:(g + 1) * P]])
            while agi < AG_CHUNKS and ag_grp[agi] == g:
                allgather(0, agi)
                agi += 1
        for l in range(1, 5):
            agi = 0
            for g in range(ngrp):
                edge(l, g)
                if l < 4:
                    while agi < AG_CHUNKS and ag_grp[agi] == g:
                        allgather(l, agi)
                        agi += 1

        # ---- pooling ----
        pps = gps_pool.tile([N_GRAPHS, HC], f32, tag="grp")
        for g in range(ngrp):
            mb = work.tile([P, N_GRAPHS], bf16, tag="mb")
            nc.vector.tensor_tensor(
                out=mb[:],
                in0=bat_t[:, g:g + 1].to_broadcast([P, N_GRAPHS]),
                in1=iota_t[:, 0:N_GRAPHS],
                op=Alu.is_equal,
            )
            nc.tensor.matmul(pps[:], lhsT=mb[:], rhs=pool_in[:, g, :],
                             start=(g == 0), stop=(g == ngrp - 1))
        psb = work.tile([N_GRAPHS, HC], f32, tag="psb")
        nc.vector.tensor_copy(psb[:], pps[:])
        nc.sync.dma_start(out=ar_in[:], in_=psb[:])
        nc.gpsimd.collective_compute(
            "AllReduce", Alu.add, replica_groups=RG,
            ins=[ar_in[:].opt()], outs=[ar_out[:].opt()],
        )
        ps2 = work.tile([N_GRAPHS, HC], f32, tag="ps2")
        nc.sync.dma_start(out=ps2[:], in_=ar_out[:])
        hg = work.tile([N_GRAPHS, HC], f32, tag="hg")
        nc.vector.tensor_scalar_mul(hg[:], ps2[:, 0:HC], rcinv_t[:, 0:1])
        hgb = work.tile([N_GRAPHS, HC], bf16, tag="hgb")
        nc.scalar.activation(hgb[:], hg[:], Act.Copy)
        # MLP layer 1
        nks = _ceil(HC, P)
        z1p = psum.tile([N_GRAPHS, 32], f32, tag="dens")
        hgT = []
        for ks in range(nks):
            ksz = min(P, HC - ks * P)
            tp = psum.tile([P, N_GRAPHS], bf16, tag="transp")
            nc.tensor.transpose(tp[:ksz, :], hgb[:, ks * P:ks * P + ksz],
                                ident_t[:N_GRAPHS, :N_GRAPHS])
            ht = work.tile([P, N_GRAPHS], bf16, tag=f"hgT{ks}")
            nc.scalar.activation(ht[:ksz, :], tp[:ksz, :], Act.Copy)
            hgT.append(ht)
        for ks in range(nks):
            ksz = min(P, HC - ks * P)
            nc.tensor.matmul(z1p[:], lhsT=hgT[ks][:ksz, :], rhs=wm1_t[ks][:],
                             start=(ks == 0), stop=(ks == nks - 1))
        z1 = work.tile([N_GRAPHS, 32], f32, tag="z1s")
        nc.vector.tensor_tensor(out=z1[:], in0=z1p[:], in1=bm1_t[:],
                                op=Alu.add)
        z1b = work.tile([N_GRAPHS, 32], bf16, tag="z1b")
        nc.scalar.activation(z1b[:], z1[:], Act.Relu)
        # MLP layer 2
        tp2 = psum.tile([32, N_GRAPHS], bf16, tag="transp")
        nc.tensor.transpose(tp2[:], z1b[:], ident_t[:N_GRAPHS, :N_GRAPHS])
        z1T = work.tile([32, N_GRAPHS], bf16, tag="z1Ts")
        nc.scalar.activation(z1T[:], tp2[:], Act.Copy)
        z2p = psum.tile([N_GRAPHS, 1], f32, tag="dens")
        nc.tensor.matmul(z2p[:], lhsT=z1T[:], rhs=wm2_t[:], start=True,
                         stop=True)
        ob = work.tile([N_GRAPHS, 1], f32, tag="ob")
        nc.vector.tensor_tensor(out=ob[:], in0=z2p[:], in1=bm2_t[:],
                                op=Alu.add)
        nc.scalar.activation(ob[:], ob[:], Act.Sigmoid)
        nc.sync.dma_start(out=t_out[:], in_=ob[:])

    nc.compile()
    return nc


# ----------------------------------------------------------------------------
# entry point
# ----------------------------------------------------------------------------

def kernel(**inputs):
    import concourse.bass_utils as bass_utils

    cfg = FULL
    meta, in_maps = preprocess(inputs, cfg["N"], cfg["FIN"], cfg["H"],
                               cfg["C"], cfg["n_cores"])
    nc = build_bass(meta)
    res = bass_utils.run_bass_kernel_spmd(
        nc, in_maps, core_ids=list(range(cfg["n_cores"])))
    return np.asarray(res.results[0]["out"], dtype=np.float32)
